# revision 2
# baseline (speedup 1.0000x reference)
"""Trainium2 Bass kernel for per-sample dynamic conv (SE-attention mixed 3x3 kernels).

Computation (per sample b):
    att[b, :]  = sigmoid(gn(mish(gn(mean_hw(x[b]) @ w1.T)) @ w2.T))   # [K]
    agg_w[b]   = sum_k att[b,k] * weight[k]                           # [C,C,3,3]
    agg_b[b]   = att[b, :] @ bias                                     # [C]
    out[b]     = conv2d(x[b], agg_w[b], padding=1) + agg_b[b]

Strategy: pure data parallel over batch on 8 NeuronCores (4 samples each).
Conv is 9 shifted bf16 matmuls per 8-row output block accumulating in PSUM;
x is staged host-side zero-padded to 66-wide rows so the shifts never need
edge fixups.

v2 schedule (vs the earlier baseline):
  - ALL input DMAs go on the single sync HWDGE ring in priority order
    (blob, x0 in 3 chunks, wt in 3 agg-chunks, x1..x3). Measured: one ring
    streams ~300+ GB/s; splitting a transfer across rings is SLOWER
    (they share the 16 SDMA engines and multiply fixed costs).
  - Outputs go exclusively on the scalar HWDGE ring.
  - PE warmup: ~28 dummy matmuls on the blob while x0 loads, so HAM is at
    2.4 GHz when the conv stream starts.
  - x0 pooling overlaps its own DMA (per-chunk partials, merged by the SE
    h1 matmul accumulation).
  - Per-sample SE chains (rows=1, so GN scale/bias need no partition
    broadcast); rsqrt via integer bit-hack + 1 Newton step on DVE instead
    of a 4-iteration reciprocal Newton chain.
"""

import os
import numpy as np
import ml_dtypes

BS, C, HH, WW = 32, 128, 64, 64
K, HID = 4, 8
N_CORES = 8
BSL = BS // N_CORES          # samples per core
LROW = WW + 2                # 66: row pitch with left/right zero pad
LPAD = HH * LROW + 2         # 4226: padded per-channel length
NTAP = 9
RB = 8                       # output rows per PSUM block
NBLK = HH // RB              # 8 blocks
W1 = NTAP * C                # 1152
CW = W1 // 3                 # 384: one 3-tap agg chunk
EPS = 1e-5
BLOB_W = 128                 # packed small-param blob width (f32 columns)
NWARM = 28                   # PE warmup matmuls

_cache = {}

# exec time (ns) of the last hardware run, when tracing was enabled
LAST_EXEC_NS = None


def _install_trace_hook():
    """Make run_bass_kernel_spmd(trace=True) work under axon by supplying the
    missing antenv.axon_hooks module (NTFF profiling via libaxon ctypes)."""
    import sys, types
    if "antenv.axon_hooks" in sys.modules:
        return True
    try:
        from trn_agent_boot.trn_boot import _ntff_profile_via_ctypes
        hook = _ntff_profile_via_ctypes("/opt/axon/libaxon_pjrt.so")
    except Exception:
        return False
    m = types.ModuleType("antenv.axon_hooks")
    m.get_axon_ntff_profile_hook = lambda: hook
    m.set_axon_ntff_profile_hook = lambda h: None
    sys.modules["antenv.axon_hooks"] = m
    return True


def _build_nc():
    import concourse.bass as bass
    import concourse.tile as tile
    from concourse import bacc, mybir

    f32 = mybir.dt.float32
    bf16 = mybir.dt.bfloat16
    i32 = mybir.dt.int32
    Alu = mybir.AluOpType
    Act = mybir.ActivationFunctionType

    nc = bacc.Bacc("TRN2", target_bir_lowering=False, debug=False,
                   enable_asserts=True, num_devices=N_CORES)

    xp_d = nc.dram_tensor("xp", [BSL, C, LPAD], bf16, kind="ExternalInput").ap()
    wt_d = nc.dram_tensor("wt", [K, C, W1], bf16, kind="ExternalInput").ap()
    blob_d = nc.dram_tensor("blob", [C, BLOB_W], f32, kind="ExternalInput").ap()
    out_d = nc.dram_tensor("out", [BSL, C, HH * WW], bf16, kind="ExternalOutput").ap()

    with tile.TileContext(nc) as tc:
        with (
            tc.tile_pool(name="xin", bufs=BSL) as xpool,
            tc.tile_pool(name="wts", bufs=1) as wpool,
            tc.tile_pool(name="small", bufs=1) as spool,
            tc.tile_pool(name="agg", bufs=1) as apool,
            tc.tile_pool(name="ostage", bufs=4) as opool,
            tc.tile_pool(name="psmall", bufs=2, space="PSUM") as pspool,
            tc.tile_pool(name="pconv", bufs=6, space="PSUM") as cpool,
        ):
            x_t = [xpool.tile([C, LPAD], bf16, tag=f"x{b}", name=f"xt{b}")
                   for b in range(BSL)]
            blob = spool.tile([C, BLOB_W], f32, tag="blob", name="blob")
            se1_sb = blob[:, 0:8]
            biasT_sb = blob[:, 8:12]
            gn1s_sb = blob[0:1, 16:24]
            gn1b_sb = blob[0:1, 24:32]
            gn2s_sb = blob[0:1, 32:36]
            gn2b_sb = blob[0:1, 36:40]
            se2row = blob[0:1, 40:72]
            wt_all = wpool.tile([C, K * W1], bf16)

            # ---- input DMA: single sync ring, strict priority order ----
            nc.sync.dma_start(blob[:], blob_d)
            XCH = (0, 1408, 2816, LPAD)
            for i in range(3):
                nc.sync.dma_start(x_t[0][:, XCH[i]:XCH[i + 1]],
                                  xp_d[0][:, XCH[i]:XCH[i + 1]])
            wt_src = wt_d.transpose([1, 0, 2])          # [C, K, W1] view
            wt3 = wt_all[:].rearrange("p (k w) -> p k w", k=K)
            for c in (1, 0, 2):                          # conv consumes 1,0,2
                nc.sync.dma_start(wt3[:, :, c * CW:(c + 1) * CW],
                                  wt_src[:, :, c * CW:(c + 1) * CW])
            for b in (1, 2, 3):
                nc.sync.dma_start(x_t[b][:], xp_d[b])

            # ---- PE warmup: keep HAM busy while x0 streams in ----
            warm = pspool.tile([1, 128], f32, tag="seps", name="warm")
            for _ in range(NWARM):
                nc.tensor.matmul(warm[:], lhsT=blob[:, 0:1],
                                 rhs=blob[:, 0:128], start=True, stop=True)

            # ---- pooling: pooled[ci] = sum_hw x[b, ci] as per-chunk partials
            # (zero padding doesn't affect the sum; the 1/4096 mean factor is
            # folded into se1 host-side). Partials merge inside the SE h1
            # matmul accumulation, so no extra merge op is needed. ----
            def pool_pieces(b, bounds, engines):
                pieces = []
                for i in range(len(bounds) - 1):
                    lo, hi = bounds[i], bounds[i + 1]
                    p = spool.tile([C, 1], f32, tag=f"pc{b}_{i}",
                                   name=f"pc{b}_{i}")
                    if engines[i] == "v":
                        nc.vector.tensor_reduce(out=p[:], in_=x_t[b][:, lo:hi],
                                                axis=mybir.AxisListType.X,
                                                op=Alu.add)
                    else:
                        nc.scalar.activation(x_t[b][:, lo:hi],
                                             x_t[b][:, lo:hi],
                                             Act.Identity, accum_out=p[:])
                    pieces.append(p)
                return pieces

            def gn_rows1(v_ap, n, scale_sb, bias_sb, tag):
                """GroupNorm(1) on a [1, n] vector. rstd via integer bit-hack
                seed + one Newton step (all DVE, no ACT table needed)."""
                def t(nm, w=1):
                    return spool.tile([1, w], f32, tag=f"{tag}{nm}",
                                      name=f"{tag}{nm}")
                ms = t("ms")
                nc.vector.tensor_reduce(out=ms[:], in_=v_ap,
                                        axis=mybir.AxisListType.X, op=Alu.add)
                mn = t("mn")
                nc.vector.tensor_scalar_mul(mn[:], ms[:], 1.0 / n)
                cent = t("cent", n)
                nc.vector.tensor_scalar_sub(cent[:], v_ap, mn[:, 0:1])
                sq = t("sq", n)
                nc.vector.tensor_mul(sq[:], cent[:], cent[:])
                vs = t("vs")
                nc.vector.tensor_reduce(out=vs[:], in_=sq[:],
                                        axis=mybir.AxisListType.X, op=Alu.add)
                s = t("s")
                nc.vector.tensor_scalar(s[:], vs[:], 1.0 / n, EPS,
                                        op0=Alu.mult, op1=Alu.add)
                # y0 = bits(0x5f3759df - (bits(s) >> 1)) via ~(s>>1) + magic+1
                y0 = t("y0")
                nc.vector.tensor_scalar(y0[:].bitcast(i32), s[:].bitcast(i32),
                                        1, -1, op0=Alu.logical_shift_right,
                                        op1=Alu.bitwise_xor)
                y0b = t("y0b")
                nc.vector.tensor_scalar(y0b[:].bitcast(i32),
                                        y0[:].bitcast(i32),
                                        0x5F3759E0, None, op0=Alu.add)
                sh = t("sh")
                nc.vector.tensor_scalar_mul(sh[:], s[:], -0.5)
                u = t("u")
                nc.vector.tensor_mul(u[:], y0b[:], y0b[:])
                w = t("w")
                nc.vector.tensor_scalar(w[:], u[:], sh[:, 0:1], 1.5,
                                        op0=Alu.mult, op1=Alu.add)
                r = t("r")
                nc.vector.tensor_mul(r[:], y0b[:], w[:])
                z = t("z", n)
                nc.vector.scalar_tensor_tensor(
                    out=z[:], in0=cent[:], scalar=r[:, 0:1], in1=scale_sb,
                    op0=Alu.mult, op1=Alu.mult)
                o = t("o", n)
                nc.vector.tensor_add(o[:], z[:], bias_sb)
                return o

            def se_chain1(pieces, tag):
                """SE attention for ONE sample: [C,1] pooled pieces -> att [1,K]."""
                h1 = pspool.tile([1, HID], f32, tag="seps", name=f"{tag}h1")
                for i, p in enumerate(pieces):
                    nc.tensor.matmul(h1[:], lhsT=p[:], rhs=se1_sb,
                                     start=(i == 0),
                                     stop=(i == len(pieces) - 1))
                h1n = gn_rows1(h1[:], HID, gn1s_sb, gn1b_sb, f"{tag}g1")

                def t(nm, w):
                    return spool.tile([1, w], f32, tag=f"{tag}{nm}",
                                      name=f"{tag}{nm}")
                # mish(v) = v*tanh(softplus(v)) = v*u/(u+2), u = (E+2)E, E=e^v
                ev = t("ev", HID)
                nc.scalar.activation(ev[:], h1n[:], Act.Exp)
                u2 = t("u2", HID)
                nc.vector.scalar_tensor_tensor(out=u2[:], in0=ev[:],
                                               scalar=2.0, in1=ev[:],
                                               op0=Alu.add, op1=Alu.mult)
                d = t("d", HID)
                nc.vector.tensor_scalar_add(d[:], u2[:], 2.0)
                rr = t("rr", HID)
                nc.vector.reciprocal(rr[:], d[:])
                m = t("m", HID)
                nc.vector.tensor_mul(m[:], h1n[:], u2[:])
                h1m = t("h1m", HID)
                nc.vector.tensor_mul(h1m[:], m[:], rr[:])
                # h2[k] = sum_h h1m[h] * se_w2[k, h]
                hk = t("hk", K * HID)
                nc.vector.tensor_mul(
                    hk[:].rearrange("p (k h) -> p k h", k=K),
                    h1m[:].unsqueeze(1).broadcast_to([1, K, HID]),
                    se2row.rearrange("p (k h) -> p k h", k=K))
                h2 = t("h2", K)
                nc.vector.tensor_reduce(
                    out=h2[:], in_=hk[:].rearrange("p (k h) -> p k h", k=K),
                    axis=mybir.AxisListType.X, op=Alu.add)
                h2n = gn_rows1(h2[:], K, gn2s_sb, gn2b_sb, f"{tag}g2")
                # sigmoid(z) = 0.5 * (1 + tanh(z/2))
                tnh = t("tnh", K)
                nc.scalar.activation(tnh[:], h2n[:], Act.Tanh, scale=0.5)
                att = t("att", K)
                nc.vector.tensor_scalar(att[:], tnh[:], 0.5, 0.5,
                                        op0=Alu.mult, op1=Alu.add)
                return att

            def att_setup(att, b):
                """Broadcast att to all partitions; agg_b on DVE."""
                att_bc = spool.tile([C, K], f32, tag=f"attbc{b}",
                                    name=f"attbc{b}")
                nc.gpsimd.partition_broadcast(att_bc[:], att[:])
                tmp = spool.tile([C, K], f32, tag=f"gbt{b}", name=f"gbt{b}")
                nc.vector.tensor_mul(tmp[:], biasT_sb, att_bc[:])
                aggb = spool.tile([C, 1], f32, tag=f"gb{b}", name=f"gb{b}")
                nc.vector.tensor_reduce(out=aggb[:], in_=tmp[:],
                                        axis=mybir.AxisListType.X, op=Alu.add)
                return att_bc, aggb

            def aggregate(b, att_bc, chunks):
                cw = W1 // chunks
                order = (1, 0, 2) if chunks == 3 else range(chunks)
                out_by_c = {}
                for c in order:
                    prev = None
                    for k in range(K):
                        cur = apool.tile([C, cw], bf16,
                                         tag=f"agg{b % 2}_{c}_{k}",
                                         name=f"agg{b}_{c}_{k}")
                        sc = att_bc[:, k:k + 1]
                        wk = wt_all[:, k * W1 + c * cw:k * W1 + (c + 1) * cw]
                        if prev is None:
                            nc.vector.tensor_scalar_mul(cur[:], wk, sc)
                        else:
                            nc.vector.scalar_tensor_tensor(
                                out=cur[:], in0=wk, scalar=sc, in1=prev[:],
                                op0=Alu.mult, op1=Alu.add)
                        prev = cur
                    out_by_c[c] = prev

                def agg_tap(tap):
                    c, r = divmod(tap * C, cw)
                    return out_by_c[c][:, r:r + C]
                return agg_tap

            def conv_sample(b, agg_tap, aggb):
                for blk in range(NBLK):
                    h0 = blk * RB
                    ps = cpool.tile([C, RB * WW], f32, tag="convps",
                                    name=f"cps{b}_{blk}")
                    ti = 0
                    for dh in (0, -1, 1):
                        for dw in (-1, 0, 1):
                            tt = 1 if h0 + dh < 0 else 0
                            bt = 1 if h0 + RB - 1 + dh > HH - 1 else 0
                            nr = RB - tt - bt
                            tap = (dh + 1) * 3 + (dw + 1)
                            start = 1 + (h0 + tt + dh) * LROW + dw
                            rhs = (x_t[b][:, start:start + nr * LROW]
                                   .rearrange("p (r c) -> p r c", c=LROW)
                                   [:, :, 0:WW])
                            nc.tensor.matmul(
                                ps[:, tt * WW:(tt + nr) * WW],
                                lhsT=agg_tap(tap), rhs=rhs,
                                start=(ti == 0), stop=(ti == NTAP - 1))
                            ti += 1
                    osb = opool.tile([C, RB * WW], bf16, tag="osb",
                                     name=f"osb{b}_{blk}")
                    if blk % 2 == 0:
                        nc.scalar.activation(osb[:], ps[:], Act.Identity,
                                             bias=aggb[:, 0:1], scale=1.0)
                    else:
                        nc.vector.tensor_scalar(
                            osb[:], ps[:], aggb[:, 0:1], None, op0=Alu.add)
                    dst = out_d[b][:, h0 * WW:(h0 + RB) * WW]
                    if b == BSL - 1 and blk >= NBLK - 2:
                        # drain the tail on both rings
                        nc.scalar.dma_start(dst[0:64, :], osb[0:64, :])
                        nc.sync.dma_start(dst[64:128, :], osb[64:128, :])
                    else:
                        nc.scalar.dma_start(dst, osb[:])

            # ---- schedule ----
            pieces0 = pool_pieces(0, XCH, ("a", "v", "a"))
            att0 = se_chain1(pieces0, "s0")
            att_bc0, aggb0 = att_setup(att0, 0)
            at0 = aggregate(0, att_bc0, chunks=3)
            conv_sample(0, at0, aggb0)

            HALF = LPAD // 2
            for b in (1, 2, 3):
                pieces = pool_pieces(b, (0, HALF, LPAD), ("a", "v"))
                att = se_chain1(pieces, f"s{b}")
                att_bc, aggb = att_setup(att, b)
                at = aggregate(b, att_bc, chunks=1)
                conv_sample(b, at, aggb)

    nc.compile()
    return nc


def _stage_inputs(x, weight, bias, se_w1, gn1_scale, gn1_bias, se_w2,
                  gn2_scale, gn2_bias):
    """Host-side layout staging: shard, pad, transpose, cast. Returns in_maps."""
    bf16 = ml_dtypes.bfloat16

    # zero-padded x: per (b, ci) buffer of length LPAD; element (h, w) lives at
    # 1 + h*LROW + w, so w-1/w+64 shifts read zeros and row shifts stay in bounds.
    xp = np.zeros((BS, C, LPAD), dtype=bf16)
    xp_view = xp[:, :, 1:1 + HH * LROW].reshape(BS, C, HH, LROW)
    xp_view[:, :, :, :WW] = x.astype(bf16)

    # weight [k, o, i, h, w] -> lhsT layout [k, i, (h*3+w)*C + o]
    wt = np.ascontiguousarray(weight.transpose(0, 2, 3, 4, 1)).reshape(
        K, C, W1).astype(bf16)

    # pack all small params into one [C, BLOB_W] f32 blob (single DMA issue)
    blob = np.zeros((C, BLOB_W), dtype=np.float32)
    blob[:, 0:8] = (se_w1 / float(HH * WW)).T          # se1 [C, HID]
    blob[:, 8:12] = bias.T                             # bias.T [C, K]
    blob[0:1, 16:24] = gn1_scale.reshape(1, HID)
    blob[0:1, 24:32] = gn1_bias.reshape(1, HID)
    blob[0:1, 32:36] = gn2_scale.reshape(1, K)
    blob[0:1, 36:40] = gn2_bias.reshape(1, K)
    blob[0:1, 40:72] = se_w2.reshape(1, K * HID)       # se2 rows, flat

    in_maps = []
    for i in range(N_CORES):
        in_maps.append({
            "xp": np.ascontiguousarray(xp[i * BSL:(i + 1) * BSL]),
            "wt": wt, "blob": blob,
        })
    return in_maps


def kernel(x, weight, bias, se_w1, gn1_scale, gn1_bias, se_w2, gn2_scale,
           gn2_bias):
    global LAST_EXEC_NS
    x = np.asarray(x, dtype=np.float32)
    weight = np.asarray(weight, dtype=np.float32)
    bias = np.asarray(bias, dtype=np.float32)
    se_w1 = np.asarray(se_w1, dtype=np.float32)
    gn1_scale = np.asarray(gn1_scale, dtype=np.float32)
    gn1_bias = np.asarray(gn1_bias, dtype=np.float32)
    se_w2 = np.asarray(se_w2, dtype=np.float32)
    gn2_scale = np.asarray(gn2_scale, dtype=np.float32)
    gn2_bias = np.asarray(gn2_bias, dtype=np.float32)

    if "nc" not in _cache:
        _cache["nc"] = _build_nc()
    nc = _cache["nc"]

    in_maps = _stage_inputs(x, weight, bias, se_w1, gn1_scale, gn1_bias,
                            se_w2, gn2_scale, gn2_bias)

    trace = bool(int(os.environ.get("BASS_KERNEL_TRACE", "0")))
    if trace:
        trace = _install_trace_hook()

    from concourse.bass_utils import run_bass_kernel_spmd
    res = run_bass_kernel_spmd(nc, in_maps, core_ids=list(range(N_CORES)),
                               trace=trace)
    LAST_EXEC_NS = res.exec_time_ns

    out = np.empty((BS, C, HH, WW), dtype=np.float32)
    for i in range(N_CORES):
        out[i * BSL:(i + 1) * BSL] = (
            res.results[i]["out"].astype(np.float32).reshape(BSL, C, HH, WW))
    return out
